# revision 24
# baseline (speedup 1.0000x reference)
"""Causal self-attention with RoPE on 8 Trainium2 NeuronCores.

Sharding: tensor-parallel over heads (16 heads -> 2 per core) for
QKV projections, RoPE and attention; AllToAll re-shards the attention
output from head-sharded to token-sharded; the output projection then
runs token-parallel, so no all-reduce is needed.

The AllToAll is split six ways -- (head h) x (batch-0 | batch-1 first
half | batch-1 second half) -- with token ownership remapped so every
piece fires as soon as its attention chunks finish:
  core c owns batch-0 tokens [256c, 256c+256), batch-1 tokens
  [128c, 128c+128) and [1024+128c, 1024+128c+128).
The last collective moves only 0.25 MiB and lands while the PE is
still busy with earlier output-projection work, hiding the entire
communication cost.

Shapes (hardcoded): x [2, 2048, 2048], W_* [2048, 2048], 16 heads,
d_k = 128, fp32 in/out, bf16 on-chip.

On-chip dataflow per core (all matmuls via PE, contraction on the
partition axis):
  - xT chunks [128d x (16kb x 256t)] stream in; per head h:
      qT/kT [128dk, 256t] = sum_kb Wq_h_kb.T @ xT_kb   (PSUM)
      RoPE applied with a stream_shuffle pair-swap + 2 muls + add
  - v in natural [token, d] layout: v = x_blk @ Wv.T
  - attention works on transposed scores: ST[j*128 keys, 512 q] =
      kT_j.T @ qT_i ; p = exp(ST + causal_mask); outT += v_j.T @ p
      -- no max-subtraction needed (logits are O(1) by construction).
  - softmax denominator via transposed micro-matmuls: for each 128-q
    slice, lT[128q, 1] += p_slice.T @ ones -- output free size 1, so
    the PE streams ~0 columns (vs 512 for the ones.T @ p row-sum).
  - normalize: r = 1/l; [128,4] -> PE-transpose -> [4,128] -> 4 bcast
    matmuls -> R[128, 512]; y = outT * R
  - out projection per eb feature block: 16-step PSUM accumulation
    over all 16 y row-blocks, one pass per ownership segment.
"""

import sys

for _p in ("/opt/trn_rl_repo", "/opt/pypackages"):
    if _p not in sys.path:
        sys.path.insert(0, _p)

import numpy as np

import concourse.bass as bass
import concourse.bacc as bacc
import concourse.mybir as mybir
import concourse.tile as tile
from concourse import bass_utils
from concourse.alu_op_type import AluOpType

# ---------------------------------------------------------------- config
N_CORES = 8
B, S, D = 2, 2048, 2048
H = 16
DK = D // H              # 128
HPC = H // N_CORES       # 2 heads per core
TOK = B * S              # 4096
SUB = 256                # token sub-chunk for projections
QCH = 512                # attention query chunk
JB = 128                 # attention key block
NSUB = TOK // SUB        # 16
KB = D // 128            # 16 contraction blocks
ROPE_BASE = 10000.0
MASK_NEG = -30000.0
SEGW = (256, 128, 128)   # ownership segment widths (b0, b1a, b1b)

F32 = mybir.dt.float32
BF16 = mybir.dt.bfloat16
L_TRANSPOSED = False      # False: row-sum l via ones.T @ p (slower, simple)


def _np_dt():
    import ml_dtypes
    return np.dtype(ml_dtypes.bfloat16)


# ---------------------------------------------------------------- build
_CACHE = {}


def _build_nc(repeat=1):
    dt = BF16
    nc = bacc.Bacc("TRN2", target_bir_lowering=False, debug=False,
                   num_devices=N_CORES)

    xT = nc.dram_tensor("xT", [D, TOK], dt, kind="ExternalInput")
    wqT = nc.dram_tensor("wqT", [D, HPC * DK], dt, kind="ExternalInput")
    wkT = nc.dram_tensor("wkT", [D, HPC * DK], dt, kind="ExternalInput")
    wvT = nc.dram_tensor("wvT", [D, HPC * DK], dt, kind="ExternalInput")
    # eb-major Wo: woE[eb*128+p, dl*128+c] = W_o[eb*128+c, dl*128+p]
    woE = nc.dram_tensor("woE", [D, D], dt, kind="ExternalInput")
    ropeC = nc.dram_tensor("ropeC", [DK, S], F32, kind="ExternalInput")
    ropeS = nc.dram_tensor("ropeS", [DK, S], F32, kind="ExternalInput")
    maskd = nc.dram_tensor("maskd", [JB, 4 * QCH], F32, kind="ExternalInput")
    ident = nc.dram_tensor("ident", [128, 128], dt, kind="ExternalInput")
    outT = nc.dram_tensor("outT", [D, QCH], F32, kind="ExternalOutput")

    swap_mask = [i ^ 1 for i in range(32)]

    import contextlib
    with tile.TileContext(nc) as tc:
      for _rep in range(repeat):
        with contextlib.ExitStack() as st_outer:
            dram = st_outer.enter_context(
                tc.tile_pool(name="dram", bufs=1, space="DRAM"))
            y_a2a = [[dram.tile([N_CORES * 128, w], dt,
                                name=f"y_a2a{h}_{s}")
                      for s, w in enumerate(SEGW)] for h in range(HPC)]
            yfull = [[dram.tile([N_CORES * 128, w], dt,
                                name=f"yfull{h}_{s}")
                      for s, w in enumerate(SEGW)] for h in range(HPC)]

            const = st_outer.enter_context(tc.tile_pool(name="const", bufs=1))
            # Wo + yT pools live alongside the pass pools (no address
            # reuse), so Wo prefetch can run during the head-1 pass.
            wopool = st_outer.enter_context(
                tc.tile_pool(name="wopool", bufs=1))
            ytpool = st_outer.enter_context(
                tc.tile_pool(name="ytpool", bufs=1))
            st_xq = st_outer.enter_context(contextlib.ExitStack())
            xpool = st_xq.enter_context(
                tc.tile_pool(name="xpool", bufs=3, side="right"))
            qpool = st_xq.enter_context(
                tc.tile_pool(name="qpool", bufs=4, side="right"))
            kvpool = st_xq.enter_context(
                tc.tile_pool(name="kvpool", bufs=8, side="right"))
            vpool = st_xq.enter_context(
                tc.tile_pool(name="vpool", bufs=32, side="right"))
            work = st_outer.enter_context(tc.tile_pool(name="work", bufs=2))
            ppool = st_outer.enter_context(tc.tile_pool(name="ppool", bufs=2))
            ps_proj = st_outer.enter_context(
                tc.tile_pool(name="ps_proj", bufs=2, space="PSUM"))
            ps_st = st_outer.enter_context(
                tc.tile_pool(name="ps_st", bufs=2, space="PSUM"))
            ps_out = st_outer.enter_context(
                tc.tile_pool(name="ps_out", bufs=1, space="PSUM"))
            ps_misc = st_outer.enter_context(
                tc.tile_pool(name="ps_misc", bufs=1, space="PSUM"))
            ps_lt = st_outer.enter_context(
                tc.tile_pool(name="ps_lt", bufs=1, space="PSUM"))

            # chunk-0 xT goes first on its queues so the first
            # projection isn't stuck behind weight DMAs
            def xt_dma(xt, sc):
                KH = KB // 4
                for xh in range(4):
                    eng = nc.sync if xh % 2 == 0 else nc.scalar
                    eng.dma_start(
                        xt[:, xh * KH * SUB:(xh + 1) * KH * SUB]
                          .rearrange("p (kb t) -> p kb t", kb=KH),
                        xT.ap()[xh * KH * 128:(xh + 1) * KH * 128,
                                sc * SUB:(sc + 1) * SUB]
                          .rearrange("(kb p) t -> p kb t", p=128))

            xt0 = xpool.tile([128, KB * SUB], dt, tag="xt", name="xt")
            xt_dma(xt0, 0)

            # ---- persistent constants in SBUF
            # weight DMAs split in groups, spread over the three HWDGE
            # queues, so the first projection matmuls start early
            wq_sb = const.tile([128, KB * HPC * DK], dt)
            wk_sb = const.tile([128, KB * HPC * DK], dt)
            wv_sb = const.tile([128, KB * HPC * DK], dt)
            weng = {0: nc.scalar, 1: nc.sync}
            for ti, (sb_t, dr) in enumerate(
                    ((wq_sb, wqT), (wk_sb, wkT), (wv_sb, wvT))):
                ngrp = 8 if ti == 0 else 4
                GW = KB // ngrp
                for g in range(ngrp):
                    m0 = g * GW * HPC * DK
                    weng[(ti + g) % 2].dma_start(
                        sb_t[:, m0:m0 + GW * HPC * DK]
                            .rearrange("p (kb m) -> p kb m", kb=GW),
                        dr.ap()[g * GW * 128:(g + 1) * GW * 128, :]
                          .rearrange("(kb p) m -> p kb m", p=128))
            ropeC_sb = const.tile([DK, S], F32)
            ropeS_sb = const.tile([DK, S], F32)
            maskd_sb = const.tile([JB, 4 * QCH], F32)
            ident_sb = const.tile([128, 128], dt)
            nc.scalar.dma_start(ropeC_sb[:], ropeC[:])
            nc.sync.dma_start(ropeS_sb[:], ropeS[:])
            nc.scalar.dma_start(maskd_sb[:], maskd[:])
            nc.sync.dma_start(ident_sb[:], ident[:])
            ones_col_f32 = const.tile([128, 1], F32)
            ones_row_f32 = const.tile([1, 128], F32)
            nc.vector.memset(ones_col_f32[:], 1.0)
            nc.vector.memset(ones_row_f32[:], 1.0)
            ones_col = const.tile([128, 1], dt)
            ones_row = const.tile([1, 128], dt)
            nc.vector.tensor_copy(ones_col[:], ones_col_f32[:])
            nc.vector.tensor_copy(ones_row[:], ones_row_f32[:])

            v_tiles = {}
            wo_tiles = [None] * KB

            def rope_combine(ps_in, out_ap, s0, n):
                """out = ps_in * C + shuffle(ps_in) * S  (RoPE)."""
                qsh = work.tile([128, SUB], F32, tag="qsh")
                t1 = work.tile([128, SUB], F32, tag="t1")
                nc.vector.stream_shuffle(qsh[:, :n], ps_in, swap_mask)
                nc.any.tensor_tensor(
                    t1[:, :n], ps_in, ropeC_sb[:, s0:s0 + n], AluOpType.mult)
                nc.vector.tensor_tensor(
                    qsh[:, :n], qsh[:, :n], ropeS_sb[:, s0:s0 + n],
                    AluOpType.mult)
                nc.any.tensor_tensor(out_ap, t1[:, :n], qsh[:, :n],
                                     AluOpType.add)

            # ================= two passes over the sequence, one per head
            for h in range(HPC):
                qT_tile = [None]
                kT_tiles = {}
                for sc in range(NSUB):
                    b = sc // (NSUB // B)
                    s0 = (sc % (NSUB // B)) * SUB   # position within batch
                    half = sc % 2
                    i_q = (sc % (NSUB // B)) // 2   # query chunk in batch

                    if h == 0 and sc == 0:
                        xt = xt0
                    else:
                        xt = xpool.tile([128, KB * SUB], dt, tag="xt",
                                        name="xt")
                        xt_dma(xt, sc)

                    # Wo prefetch spread across the head-1 pass; used by
                    # all three out-projection passes afterwards
                    if h == 1:
                        wo_eb = wopool.tile([128, KB * 128], dt, tag=f"wo{sc}",
                                            name="wo_eb")
                        nc.gpsimd.dma_start(
                            wo_eb[:], woE.ap()[sc * 128:(sc + 1) * 128, :])
                        wo_tiles[sc] = wo_eb

                    # ---- q/k projections + rope for this head
                    if half == 0:
                        qT_tile[0] = qpool.tile([128, QCH], dt, tag="qT",
                                                name="qT")
                    if (b, i_q) not in kT_tiles:
                        kT_tiles[(b, i_q)] = kvpool.tile(
                            [128, QCH], dt, tag="kT", name="kT")
                    for (w_sb, dst) in ((wq_sb, qT_tile[0]),
                                        (wk_sb, kT_tiles[(b, i_q)])):
                        psq = ps_proj.tile([128, SUB], F32, tag="proj")
                        for kb in range(KB):
                            nc.tensor.matmul(
                                psq[:],
                                w_sb[:, kb * HPC * DK + h * DK:
                                     kb * HPC * DK + (h + 1) * DK],
                                xt[:, kb * SUB:(kb + 1) * SUB],
                                start=(kb == 0), stop=(kb == KB - 1))
                        rope_combine(psq[:],
                                     dst[:, half * SUB:(half + 1) * SUB],
                                     s0, SUB)

                    # ---- v projection: both heads at once, pass 0 only
                    if h == 0:
                        for tb in range(SUB // 128):
                            jb_b = (sc % (NSUB // B)) * 2 + tb
                            psv = ps_proj.tile([128, HPC * DK], F32,
                                               tag="proj", name="psv")
                            for kb in range(KB):
                                nc.tensor.matmul(
                                    psv[:],
                                    xt[:, kb * SUB + tb * 128:
                                       kb * SUB + (tb + 1) * 128],
                                    wv_sb[:, kb * HPC * DK:
                                          (kb + 1) * HPC * DK],
                                    start=(kb == 0), stop=(kb == KB - 1))
                            vt = vpool.tile([128, HPC * DK], dt, tag="v",
                                            name="vt")
                            nc.vector.tensor_copy(vt[:], psv[:])
                            v_tiles[(b, jb_b)] = vt

                    # ---- attention for the completed query chunk
                    if half != 1:
                        continue
                    n_j = 4 * i_q + 4
                    qT = qT_tile[0]
                    ps_o = ps_out.tile([128, QCH], F32, tag="att_out")
                    # all four 128-q-slice denominators live in one PSUM
                    # bank as four column accumulation groups
                    if L_TRANSPOSED:
                        ps_l4 = ps_lt.tile([128, 4], F32, tag="lt",
                                           name="ps_l4")
                    else:
                        ps_lrow = ps_lt.tile([1, QCH], F32, tag="lrow",
                                             name="ps_lrow")
                    for j in range(n_j):
                        jc, jr = j // 4, j % 4
                        # diagonal blocks with offset m have their first
                        # 128*m query columns fully masked: shrink all the
                        # work to the valid column range. j == 0 is always
                        # full width, so it opens the PSUM groups.
                        m = j - 4 * i_q
                        q0 = 128 * m if m > 0 else 0
                        ps_s = ps_st.tile([JB, QCH], F32, tag="st")
                        nc.tensor.matmul(
                            ps_s[:, q0:],
                            kT_tiles[(b, jc)][:, jr * 128:(jr + 1) * 128],
                            qT[:, q0:],
                            start=True, stop=True)
                        p_t = ppool.tile([JB, QCH], dt, tag="p")
                        if m >= 0:                 # diagonal block: mask
                            nc.vector.tensor_tensor(
                                ps_s[:, q0:], ps_s[:, q0:],
                                maskd_sb[:, m * QCH + q0:(m + 1) * QCH],
                                AluOpType.add)
                        nc.scalar.activation(
                            p_t[:, q0:], ps_s[:, q0:],
                            mybir.ActivationFunctionType.Exp)
                        # transposed denominator accumulation: per 128-q
                        # slice, lT[128q, 1] += p_slice.T @ ones (free
                        # size 1 -> ~0 PE cycles)
                        if L_TRANSPOSED:
                            # one bank-wide accumulation group: start=True
                            # zeroes the whole 2KB PSUM bank, so only the
                            # first micro-matmul starts and only the very
                            # last one stops
                            for qs in range(max(m, 0), 4):
                                nc.tensor.matmul(
                                    ps_l4[:, qs:qs + 1],
                                    p_t[:, qs * 128:(qs + 1) * 128],
                                    ones_col[:],
                                    start=(j == 0 and qs == 0),
                                    stop=(j == n_j - 1 and qs == 3))
                        else:
                            nc.tensor.matmul(
                                ps_lrow[:, q0:], ones_col[:], p_t[:, q0:],
                                start=(j == 0), stop=(j == n_j - 1))
                        nc.tensor.matmul(
                            ps_o[:, q0:],
                            v_tiles[(b, j)][:, h * DK:(h + 1) * DK],
                            p_t[:, q0:],
                            start=(j == 0), stop=(j == n_j - 1))
                    # normalize: r = 1/l; four single-column PE
                    # transposes [128,1] -> [1,128] (outputs at
                    # partition 0), then one K=1 broadcast matmul
                    r_sb = work.tile([1, QCH], dt, tag="rrow")
                    if L_TRANSPOSED:
                        lr_sb = work.tile([128, 4], dt, tag="lr")
                        with nc.allow_low_precision(
                                reason="1/l bcast in bf16; y is bf16 anyway"):
                            nc.vector.reciprocal(lr_sb[:], ps_l4[:])
                        ps_row = ps_lt.tile([1, QCH], dt, tag="row",
                                            name="ps_row")
                        for qs in range(4):
                            # start=True zeroes the whole bank: only the
                            # first transpose starts, only the last stops
                            nc.tensor.matmul(ps_row[:, qs * 128:
                                                    (qs + 1) * 128],
                                             lr_sb[:, qs:qs + 1],
                                             ident_sb[:],
                                             is_transpose=True,
                                             start=(qs == 0),
                                             stop=(qs == 3))
                        nc.any.tensor_copy(r_sb[:], ps_row[:])
                    else:
                        with nc.allow_low_precision(
                                reason="1/l bcast in bf16; y is bf16 anyway"):
                            nc.vector.reciprocal(r_sb[:], ps_lrow[:])
                    ps_r = ps_misc.tile([128, QCH], F32, tag="R")
                    nc.tensor.matmul(ps_r[:], ones_row[:], r_sb[:],
                                     start=True, stop=True)
                    r_bc = work.tile([128, QCH], F32, tag="rbc")
                    nc.any.tensor_copy(r_bc[:], ps_r[:])
                    y_sb = work.tile([128, QCH], dt, tag="y")
                    nc.any.tensor_tensor(y_sb[:], ps_o[:], r_bc[:],
                                         AluOpType.mult)
                    # stores split by token ownership segment
                    if b == 0:
                        for t in range(2):
                            blk = 2 * i_q + t
                            (nc.sync if t == 0 else nc.scalar).dma_start(
                                y_a2a[h][0][blk * 128:(blk + 1) * 128, :],
                                y_sb[:, t * 256:(t + 1) * 256])
                    else:
                        seg = 1 if i_q < 2 else 2
                        for t in range(4):
                            blk = 4 * (i_q % 2) + t
                            (nc.sync if t % 2 == 0 else nc.scalar).dma_start(
                                y_a2a[h][seg][blk * 128:(blk + 1) * 128, :],
                                y_sb[:, t * 128:(t + 1) * 128])
                    # fire the segment's AllToAll once its chunks are done
                    done_seg = (0 if (b, i_q) == (0, 3) else
                                1 if (b, i_q) == (1, 1) else
                                2 if (b, i_q) == (1, 3) else None)
                    if done_seg is not None:
                        nc.gpsimd.collective_compute(
                            "AllToAll", AluOpType.bypass,
                            replica_groups=[list(range(N_CORES))],
                            ins=[y_a2a[h][done_seg].opt()],
                            outs=[yfull[h][done_seg].opt()])

            # x/q/kv/v pools are dead now; the out-projection reads the
            # AllToAll results (token-sharded y) against the resident Wo
            st_xq.close()

            yT = [[None] * 3 for _ in range(HPC)]
            yld = [nc.sync, nc.scalar]
            for s, w in enumerate(SEGW):
                for hh in range(HPC):
                    yt = ytpool.tile([128, N_CORES * w], dt,
                                     tag=f"yt{hh}_{s}", name="yt")
                    yld[(2 * s + hh) % 2].dma_start(
                        yt.rearrange("p (s t) -> p s t", s=N_CORES),
                        yfull[hh][s][:]
                        .rearrange("(s p) t -> p s t", p=128))
                    yT[hh][s] = yt

            # ---- output projection, one pass per ownership segment
            col0 = 0
            for s, w in enumerate(SEGW):
                for eb in range(KB):
                    ps_w = ps_st.tile([JB, QCH], F32, tag="st", name="ps_w")
                    for dl in range(KB):
                        src, hh = dl // 2, dl % 2
                        nc.tensor.matmul(
                            ps_w[:, :w],
                            wo_tiles[eb][:, dl * 128:(dl + 1) * 128],
                            yT[hh][s][:, src * w:(src + 1) * w],
                            start=(dl == 0), stop=(dl == KB - 1))
                    o_sb = work.tile([128, QCH], F32, tag="y")
                    nc.any.tensor_copy(o_sb[:, :w], ps_w[:, :w])
                    yld[eb % 2].dma_start(
                        outT[eb * 128:(eb + 1) * 128, col0:col0 + w],
                        o_sb[:, :w])
                col0 += w

    nc.finalize()
    return nc


# ---------------------------------------------------------------- host
def _host_inputs(x, W_q, W_k, W_v, W_o):
    np_dt = _np_dt()
    xT = np.ascontiguousarray(
        x.reshape(TOK, D).T).astype(np_dt)                     # [D, TOK]
    # eb-major Wo for contiguous per-eb DMA rows:
    # woE[eb*128+p, dl*128+c] = W_o[eb*128+c, dl*128+p]
    woE = np.ascontiguousarray(
        W_o.reshape(KB, 128, KB, 128).transpose(0, 3, 2, 1)
        .reshape(D, D)).astype(np_dt)

    # RoPE tables, expanded to [DK, S] with interleaved pairs; the sign
    # table carries -sin on even rows, +sin on odd rows.
    i = np.arange(0, DK, 2, dtype=np.float32)
    theta = 1.0 / (ROPE_BASE ** (i / DK))                      # [64]
    pos = np.arange(S, dtype=np.float32)
    freqs = pos[:, None] * theta[None, :]                      # [S, 64]
    cos_t, sin_t = np.cos(freqs), np.sin(freqs)
    ropeC = np.empty((DK, S), np.float32)
    ropeS = np.empty((DK, S), np.float32)
    ropeC[0::2] = cos_t.T
    ropeC[1::2] = cos_t.T
    ropeS[0::2] = -sin_t.T
    ropeS[1::2] = sin_t.T

    # diagonal causal masks: block m (of the 4 key blocks overlapping a
    # 512-query chunk) keeps kk <= qq - 128*m
    kk = np.arange(JB)[:, None]
    qq = np.arange(QCH)[None, :]
    maskd = np.concatenate(
        [np.where(kk <= qq - 128 * m, 0.0, MASK_NEG).astype(np.float32)
         for m in range(4)], axis=1)                           # [128, 4*512]

    ident = np.eye(128, dtype=np.float32).astype(np_dt)

    scale = 1.0 / np.sqrt(np.float32(DK))
    in_maps = []
    for c in range(N_CORES):
        rows = slice(c * HPC * DK, (c + 1) * HPC * DK)
        in_maps.append({
            "xT": xT,
            "wqT": np.ascontiguousarray((W_q[rows] * scale).T).astype(np_dt),
            "wkT": np.ascontiguousarray(W_k[rows].T).astype(np_dt),
            "wvT": np.ascontiguousarray(W_v[rows].T).astype(np_dt),
            "woE": woE,
            "ropeC": ropeC,
            "ropeS": ropeS,
            "maskd": maskd,
            "ident": ident,
        })
    return in_maps


def kernel(x, W_q, W_k, W_v, W_o):
    x = np.asarray(x, dtype=np.float32)
    W_q = np.asarray(W_q, dtype=np.float32)
    W_k = np.asarray(W_k, dtype=np.float32)
    W_v = np.asarray(W_v, dtype=np.float32)
    W_o = np.asarray(W_o, dtype=np.float32)

    if "nc" not in _CACHE:
        _CACHE["nc"] = _build_nc()
    nc = _CACHE["nc"]

    in_maps = _host_inputs(x, W_q, W_k, W_v, W_o)
    res = bass_utils.run_bass_kernel_spmd(
        nc, in_maps, core_ids=list(range(N_CORES)))

    # outT per core: [D, 512] fp32; columns = [b0 256 | b1a 128 | b1b 128]
    out = np.empty((B, S, D), np.float32)
    for c in range(N_CORES):
        oT = res.results[c]["outT"]                            # [D, 512]
        out[0, c * 256:(c + 1) * 256] = oT[:, 0:256].T
        out[1, c * 128:(c + 1) * 128] = oT[:, 256:384].T
        out[1, 1024 + c * 128:1024 + (c + 1) * 128] = oT[:, 384:512].T
    return out


# revision 25
# speedup vs baseline: 1.0424x; 1.0424x over previous
"""Causal self-attention with RoPE on 8 Trainium2 NeuronCores.

Sharding: tensor-parallel over heads (16 heads -> 2 per core) for
QKV projections, RoPE and attention; AllToAll re-shards the attention
output from head-sharded to token-sharded; the output projection then
runs token-parallel, so no all-reduce is needed.

The AllToAll is split six ways -- (head h) x (batch-0 | batch-1 first
half | batch-1 second half) -- with token ownership remapped so every
piece fires as soon as its attention chunks finish:
  core c owns batch-0 tokens [256c, 256c+256), batch-1 tokens
  [128c, 128c+128) and [1024+128c, 1024+128c+128).
The last collective moves only 0.25 MiB and lands while the PE is
still busy with earlier output-projection work, hiding the entire
communication cost.

Shapes (hardcoded): x [2, 2048, 2048], W_* [2048, 2048], 16 heads,
d_k = 128, fp32 in/out, bf16 on-chip.

On-chip dataflow per core (all matmuls via PE, contraction on the
partition axis):
  - xT chunks [128d x (16kb x 256t)] stream in; per head h:
      qT/kT [128dk, 256t] = sum_kb Wq_h_kb.T @ xT_kb   (PSUM)
      RoPE applied with a stream_shuffle pair-swap + 2 muls + add
  - v in natural [token, d] layout: v = x_blk @ Wv.T
  - attention works on transposed scores: ST[j*128 keys, 512 q] =
      kT_j.T @ qT_i ; p = exp(ST + causal_mask); outT += v_j.T @ p
      -- no max-subtraction needed (logits are O(1) by construction).
  - softmax denominator via transposed micro-matmuls: for each 128-q
    slice, lT[128q, 1] += p_slice.T @ ones -- output free size 1, so
    the PE streams ~0 columns (vs 512 for the ones.T @ p row-sum).
  - normalize: r = 1/l; [128,4] -> PE-transpose -> [4,128] -> 4 bcast
    matmuls -> R[128, 512]; y = outT * R
  - out projection per eb feature block: 16-step PSUM accumulation
    over all 16 y row-blocks, one pass per ownership segment.
"""

import sys

for _p in ("/opt/trn_rl_repo", "/opt/pypackages"):
    if _p not in sys.path:
        sys.path.insert(0, _p)

import numpy as np

import concourse.bass as bass
import concourse.bacc as bacc
import concourse.mybir as mybir
import concourse.tile as tile
from concourse import bass_utils
from concourse.alu_op_type import AluOpType

# ---------------------------------------------------------------- config
N_CORES = 8
B, S, D = 2, 2048, 2048
H = 16
DK = D // H              # 128
HPC = H // N_CORES       # 2 heads per core
TOK = B * S              # 4096
SUB = 256                # token sub-chunk for projections
QCH = 512                # attention query chunk
JB = 128                 # attention key block
NSUB = TOK // SUB        # 16
KB = D // 128            # 16 contraction blocks
ROPE_BASE = 10000.0
MASK_NEG = -30000.0
SEGW = (256, 128, 128)   # ownership segment widths (b0, b1a, b1b)

F32 = mybir.dt.float32
BF16 = mybir.dt.bfloat16
L_TRANSPOSED = True      # False: row-sum l via ones.T @ p (slower, simple)


def _np_dt():
    import ml_dtypes
    return np.dtype(ml_dtypes.bfloat16)


# ---------------------------------------------------------------- build
_CACHE = {}


def _build_nc(repeat=1):
    dt = BF16
    nc = bacc.Bacc("TRN2", target_bir_lowering=False, debug=False,
                   num_devices=N_CORES)

    xT = nc.dram_tensor("xT", [D, TOK], dt, kind="ExternalInput")
    wqT = nc.dram_tensor("wqT", [D, HPC * DK], dt, kind="ExternalInput")
    wkT = nc.dram_tensor("wkT", [D, HPC * DK], dt, kind="ExternalInput")
    wvT = nc.dram_tensor("wvT", [D, HPC * DK], dt, kind="ExternalInput")
    # eb-major Wo: woE[eb*128+p, dl*128+c] = W_o[eb*128+c, dl*128+p]
    woE = nc.dram_tensor("woE", [D, D], dt, kind="ExternalInput")
    ropeC = nc.dram_tensor("ropeC", [DK, S], F32, kind="ExternalInput")
    ropeS = nc.dram_tensor("ropeS", [DK, S], F32, kind="ExternalInput")
    maskd = nc.dram_tensor("maskd", [JB, 4 * QCH], F32, kind="ExternalInput")
    ident = nc.dram_tensor("ident", [128, 128], dt, kind="ExternalInput")
    outT = nc.dram_tensor("outT", [D, QCH], F32, kind="ExternalOutput")

    swap_mask = [i ^ 1 for i in range(32)]

    import contextlib
    with tile.TileContext(nc) as tc:
      for _rep in range(repeat):
        with contextlib.ExitStack() as st_outer:
            dram = st_outer.enter_context(
                tc.tile_pool(name="dram", bufs=1, space="DRAM"))
            y_a2a = [[dram.tile([N_CORES * 128, w], dt,
                                name=f"y_a2a{h}_{s}")
                      for s, w in enumerate(SEGW)] for h in range(HPC)]
            yfull = [[dram.tile([N_CORES * 128, w], dt,
                                name=f"yfull{h}_{s}")
                      for s, w in enumerate(SEGW)] for h in range(HPC)]

            const = st_outer.enter_context(tc.tile_pool(name="const", bufs=1))
            # Wo + yT pools live alongside the pass pools (no address
            # reuse), so Wo prefetch can run during the head-1 pass.
            wopool = st_outer.enter_context(
                tc.tile_pool(name="wopool", bufs=1))
            ytpool = st_outer.enter_context(
                tc.tile_pool(name="ytpool", bufs=1))
            st_xq = st_outer.enter_context(contextlib.ExitStack())
            xpool = st_xq.enter_context(
                tc.tile_pool(name="xpool", bufs=3, side="right"))
            qpool = st_xq.enter_context(
                tc.tile_pool(name="qpool", bufs=4, side="right"))
            kvpool = st_xq.enter_context(
                tc.tile_pool(name="kvpool", bufs=8, side="right"))
            vpool = st_xq.enter_context(
                tc.tile_pool(name="vpool", bufs=32, side="right"))
            work = st_outer.enter_context(tc.tile_pool(name="work", bufs=2))
            ppool = st_outer.enter_context(tc.tile_pool(name="ppool", bufs=2))
            ps_proj = st_outer.enter_context(
                tc.tile_pool(name="ps_proj", bufs=2, space="PSUM"))
            ps_st = st_outer.enter_context(
                tc.tile_pool(name="ps_st", bufs=2, space="PSUM"))
            ps_out = st_outer.enter_context(
                tc.tile_pool(name="ps_out", bufs=1, space="PSUM"))
            ps_misc = st_outer.enter_context(
                tc.tile_pool(name="ps_misc", bufs=1, space="PSUM"))
            ps_lt = st_outer.enter_context(
                tc.tile_pool(name="ps_lt", bufs=1, space="PSUM"))

            # chunk-0 xT goes first on its queues so the first
            # projection isn't stuck behind weight DMAs
            def xt_dma(xt, sc):
                KH = KB // 4
                for xh in range(4):
                    eng = nc.sync if xh % 2 == 0 else nc.scalar
                    eng.dma_start(
                        xt[:, xh * KH * SUB:(xh + 1) * KH * SUB]
                          .rearrange("p (kb t) -> p kb t", kb=KH),
                        xT.ap()[xh * KH * 128:(xh + 1) * KH * 128,
                                sc * SUB:(sc + 1) * SUB]
                          .rearrange("(kb p) t -> p kb t", p=128))

            xt0 = xpool.tile([128, KB * SUB], dt, tag="xt", name="xt")
            xt_dma(xt0, 0)

            # ---- persistent constants in SBUF
            # weight DMAs split in groups, spread over the three HWDGE
            # queues, so the first projection matmuls start early
            wq_sb = const.tile([128, KB * HPC * DK], dt)
            wk_sb = const.tile([128, KB * HPC * DK], dt)
            wv_sb = const.tile([128, KB * HPC * DK], dt)
            weng = {0: nc.scalar, 1: nc.sync}
            for ti, (sb_t, dr) in enumerate(
                    ((wq_sb, wqT), (wk_sb, wkT), (wv_sb, wvT))):
                ngrp = 8 if ti == 0 else 4
                GW = KB // ngrp
                for g in range(ngrp):
                    m0 = g * GW * HPC * DK
                    weng[(ti + g) % 2].dma_start(
                        sb_t[:, m0:m0 + GW * HPC * DK]
                            .rearrange("p (kb m) -> p kb m", kb=GW),
                        dr.ap()[g * GW * 128:(g + 1) * GW * 128, :]
                          .rearrange("(kb p) m -> p kb m", p=128))
            ropeC_sb = const.tile([DK, S], F32)
            ropeS_sb = const.tile([DK, S], F32)
            maskd_sb = const.tile([JB, 4 * QCH], F32)
            ident_sb = const.tile([128, 128], dt)
            nc.scalar.dma_start(ropeC_sb[:], ropeC[:])
            nc.sync.dma_start(ropeS_sb[:], ropeS[:])
            nc.scalar.dma_start(maskd_sb[:], maskd[:])
            nc.sync.dma_start(ident_sb[:], ident[:])
            ones_col_f32 = const.tile([128, 1], F32)
            ones_row_f32 = const.tile([1, 128], F32)
            nc.vector.memset(ones_col_f32[:], 1.0)
            nc.vector.memset(ones_row_f32[:], 1.0)
            ones_col = const.tile([128, 1], dt)
            ones_row = const.tile([1, 128], dt)
            nc.vector.tensor_copy(ones_col[:], ones_col_f32[:])
            nc.vector.tensor_copy(ones_row[:], ones_row_f32[:])

            v_tiles = {}
            wo_tiles = [None] * KB

            def rope_combine(ps_in, out_ap, s0, n):
                """out = ps_in * C + shuffle(ps_in) * S  (RoPE)."""
                qsh = work.tile([128, SUB], F32, tag="qsh")
                t1 = work.tile([128, SUB], F32, tag="t1")
                nc.vector.stream_shuffle(qsh[:, :n], ps_in, swap_mask)
                nc.any.tensor_tensor(
                    t1[:, :n], ps_in, ropeC_sb[:, s0:s0 + n], AluOpType.mult)
                nc.vector.tensor_tensor(
                    qsh[:, :n], qsh[:, :n], ropeS_sb[:, s0:s0 + n],
                    AluOpType.mult)
                nc.any.tensor_tensor(out_ap, t1[:, :n], qsh[:, :n],
                                     AluOpType.add)

            # ================= two passes over the sequence, one per head
            for h in range(HPC):
                qT_tile = [None]
                kT_tiles = {}
                for sc in range(NSUB):
                    b = sc // (NSUB // B)
                    s0 = (sc % (NSUB // B)) * SUB   # position within batch
                    half = sc % 2
                    i_q = (sc % (NSUB // B)) // 2   # query chunk in batch

                    if h == 0 and sc == 0:
                        xt = xt0
                    else:
                        xt = xpool.tile([128, KB * SUB], dt, tag="xt",
                                        name="xt")
                        xt_dma(xt, sc)

                    # Wo prefetch spread across the head-1 pass; used by
                    # all three out-projection passes afterwards
                    if h == 1:
                        wo_eb = wopool.tile([128, KB * 128], dt, tag=f"wo{sc}",
                                            name="wo_eb")
                        nc.gpsimd.dma_start(
                            wo_eb[:], woE.ap()[sc * 128:(sc + 1) * 128, :])
                        wo_tiles[sc] = wo_eb

                    # ---- q/k projections + rope for this head
                    if half == 0:
                        qT_tile[0] = qpool.tile([128, QCH], dt, tag="qT",
                                                name="qT")
                    if (b, i_q) not in kT_tiles:
                        kT_tiles[(b, i_q)] = kvpool.tile(
                            [128, QCH], dt, tag="kT", name="kT")
                    for (w_sb, dst) in ((wq_sb, qT_tile[0]),
                                        (wk_sb, kT_tiles[(b, i_q)])):
                        psq = ps_proj.tile([128, SUB], F32, tag="proj")
                        for kb in range(KB):
                            nc.tensor.matmul(
                                psq[:],
                                w_sb[:, kb * HPC * DK + h * DK:
                                     kb * HPC * DK + (h + 1) * DK],
                                xt[:, kb * SUB:(kb + 1) * SUB],
                                start=(kb == 0), stop=(kb == KB - 1))
                        rope_combine(psq[:],
                                     dst[:, half * SUB:(half + 1) * SUB],
                                     s0, SUB)

                    # ---- v projection: both heads at once, pass 0 only
                    if h == 0:
                        for tb in range(SUB // 128):
                            jb_b = (sc % (NSUB // B)) * 2 + tb
                            psv = ps_proj.tile([128, HPC * DK], F32,
                                               tag="proj", name="psv")
                            for kb in range(KB):
                                nc.tensor.matmul(
                                    psv[:],
                                    xt[:, kb * SUB + tb * 128:
                                       kb * SUB + (tb + 1) * 128],
                                    wv_sb[:, kb * HPC * DK:
                                          (kb + 1) * HPC * DK],
                                    start=(kb == 0), stop=(kb == KB - 1))
                            vt = vpool.tile([128, HPC * DK], dt, tag="v",
                                            name="vt")
                            nc.vector.tensor_copy(vt[:], psv[:])
                            v_tiles[(b, jb_b)] = vt

                    # ---- attention for the completed query chunk
                    if half != 1:
                        continue
                    n_j = 4 * i_q + 4
                    qT = qT_tile[0]
                    ps_o = ps_out.tile([128, QCH], F32, tag="att_out")
                    # all four 128-q-slice denominators live in one PSUM
                    # bank as four column accumulation groups
                    if L_TRANSPOSED:
                        ps_l4 = ps_lt.tile([128, 4], F32, tag="lt",
                                           name="ps_l4")
                    else:
                        ps_lrow = ps_lt.tile([1, QCH], F32, tag="lrow",
                                             name="ps_lrow")
                    for j in range(n_j):
                        jc, jr = j // 4, j % 4
                        # diagonal blocks with offset m have their first
                        # 128*m query columns fully masked: shrink all the
                        # work to the valid column range. j == 0 is always
                        # full width, so it opens the PSUM groups.
                        m = j - 4 * i_q
                        q0 = 128 * m if m > 0 else 0
                        ps_s = ps_st.tile([JB, QCH], F32, tag="st")
                        nc.tensor.matmul(
                            ps_s[:, q0:],
                            kT_tiles[(b, jc)][:, jr * 128:(jr + 1) * 128],
                            qT[:, q0:],
                            start=True, stop=True)
                        p_t = ppool.tile([JB, QCH], dt, tag="p")
                        if m >= 0:                 # diagonal block: mask
                            nc.vector.tensor_tensor(
                                ps_s[:, q0:], ps_s[:, q0:],
                                maskd_sb[:, m * QCH + q0:(m + 1) * QCH],
                                AluOpType.add)
                        nc.scalar.activation(
                            p_t[:, q0:], ps_s[:, q0:],
                            mybir.ActivationFunctionType.Exp)
                        # transposed denominator accumulation: per 128-q
                        # slice, lT[128q, 1] += p_slice.T @ ones (free
                        # size 1 -> ~0 PE cycles)
                        if L_TRANSPOSED:
                            # one bank-wide accumulation group: start=True
                            # zeroes the whole 2KB PSUM bank, so only the
                            # first micro-matmul starts and only the very
                            # last one stops
                            for qs in range(max(m, 0), 4):
                                nc.tensor.matmul(
                                    ps_l4[:, qs:qs + 1],
                                    p_t[:, qs * 128:(qs + 1) * 128],
                                    ones_col[:],
                                    start=(j == 0 and qs == 0),
                                    stop=(j == n_j - 1 and qs == 3))
                        else:
                            nc.tensor.matmul(
                                ps_lrow[:, q0:], ones_col[:], p_t[:, q0:],
                                start=(j == 0), stop=(j == n_j - 1))
                        nc.tensor.matmul(
                            ps_o[:, q0:],
                            v_tiles[(b, j)][:, h * DK:(h + 1) * DK],
                            p_t[:, q0:],
                            start=(j == 0), stop=(j == n_j - 1))
                    # normalize: r = 1/l; four single-column PE
                    # transposes [128,1] -> [1,128] (outputs at
                    # partition 0), then one K=1 broadcast matmul
                    r_sb = work.tile([1, QCH], dt, tag="rrow")
                    if L_TRANSPOSED:
                        lr_sb = work.tile([128, 4], dt, tag="lr")
                        with nc.allow_low_precision(
                                reason="1/l bcast in bf16; y is bf16 anyway"):
                            nc.vector.reciprocal(lr_sb[:], ps_l4[:])
                        ps_row = ps_lt.tile([1, QCH], dt, tag="row",
                                            name="ps_row")
                        for qs in range(4):
                            # start=True zeroes the whole bank: only the
                            # first transpose starts, only the last stops
                            nc.tensor.matmul(ps_row[:, qs * 128:
                                                    (qs + 1) * 128],
                                             lr_sb[:, qs:qs + 1],
                                             ident_sb[:],
                                             is_transpose=True,
                                             start=(qs == 0),
                                             stop=(qs == 3))
                        nc.any.tensor_copy(r_sb[:], ps_row[:])
                    else:
                        with nc.allow_low_precision(
                                reason="1/l bcast in bf16; y is bf16 anyway"):
                            nc.vector.reciprocal(r_sb[:], ps_lrow[:])
                    ps_r = ps_misc.tile([128, QCH], F32, tag="R")
                    nc.tensor.matmul(ps_r[:], ones_row[:], r_sb[:],
                                     start=True, stop=True)
                    r_bc = work.tile([128, QCH], F32, tag="rbc")
                    nc.any.tensor_copy(r_bc[:], ps_r[:])
                    y_sb = work.tile([128, QCH], dt, tag="y")
                    nc.any.tensor_tensor(y_sb[:], ps_o[:], r_bc[:],
                                         AluOpType.mult)
                    # stores split by token ownership segment
                    if b == 0:
                        for t in range(2):
                            blk = 2 * i_q + t
                            (nc.sync if t == 0 else nc.scalar).dma_start(
                                y_a2a[h][0][blk * 128:(blk + 1) * 128, :],
                                y_sb[:, t * 256:(t + 1) * 256])
                    else:
                        seg = 1 if i_q < 2 else 2
                        for t in range(4):
                            blk = 4 * (i_q % 2) + t
                            (nc.sync if t % 2 == 0 else nc.scalar).dma_start(
                                y_a2a[h][seg][blk * 128:(blk + 1) * 128, :],
                                y_sb[:, t * 128:(t + 1) * 128])
                    # fire the segment's AllToAll once its chunks are done
                    done_seg = (0 if (b, i_q) == (0, 3) else
                                1 if (b, i_q) == (1, 1) else
                                2 if (b, i_q) == (1, 3) else None)
                    if done_seg is not None:
                        nc.gpsimd.collective_compute(
                            "AllToAll", AluOpType.bypass,
                            replica_groups=[list(range(N_CORES))],
                            ins=[y_a2a[h][done_seg].opt()],
                            outs=[yfull[h][done_seg].opt()])

            # x/q/kv/v pools are dead now; the out-projection reads the
            # AllToAll results (token-sharded y) against the resident Wo
            st_xq.close()

            yT = [[None] * 3 for _ in range(HPC)]
            yld = [nc.sync, nc.scalar]
            for s, w in enumerate(SEGW):
                for hh in range(HPC):
                    yt = ytpool.tile([128, N_CORES * w], dt,
                                     tag=f"yt{hh}_{s}", name="yt")
                    yld[(2 * s + hh) % 2].dma_start(
                        yt.rearrange("p (s t) -> p s t", s=N_CORES),
                        yfull[hh][s][:]
                        .rearrange("(s p) t -> p s t", p=128))
                    yT[hh][s] = yt

            # ---- output projection, one pass per ownership segment
            col0 = 0
            for s, w in enumerate(SEGW):
                for eb in range(KB):
                    ps_w = ps_st.tile([JB, QCH], F32, tag="st", name="ps_w")
                    for dl in range(KB):
                        src, hh = dl // 2, dl % 2
                        nc.tensor.matmul(
                            ps_w[:, :w],
                            wo_tiles[eb][:, dl * 128:(dl + 1) * 128],
                            yT[hh][s][:, src * w:(src + 1) * w],
                            start=(dl == 0), stop=(dl == KB - 1))
                    o_sb = work.tile([128, QCH], F32, tag="y")
                    nc.any.tensor_copy(o_sb[:, :w], ps_w[:, :w])
                    yld[eb % 2].dma_start(
                        outT[eb * 128:(eb + 1) * 128, col0:col0 + w],
                        o_sb[:, :w])
                col0 += w

    nc.finalize()
    return nc


# ---------------------------------------------------------------- host
def _host_inputs(x, W_q, W_k, W_v, W_o):
    np_dt = _np_dt()
    xT = np.ascontiguousarray(
        x.reshape(TOK, D).T).astype(np_dt)                     # [D, TOK]
    # eb-major Wo for contiguous per-eb DMA rows:
    # woE[eb*128+p, dl*128+c] = W_o[eb*128+c, dl*128+p]
    woE = np.ascontiguousarray(
        W_o.reshape(KB, 128, KB, 128).transpose(0, 3, 2, 1)
        .reshape(D, D)).astype(np_dt)

    # RoPE tables, expanded to [DK, S] with interleaved pairs; the sign
    # table carries -sin on even rows, +sin on odd rows.
    i = np.arange(0, DK, 2, dtype=np.float32)
    theta = 1.0 / (ROPE_BASE ** (i / DK))                      # [64]
    pos = np.arange(S, dtype=np.float32)
    freqs = pos[:, None] * theta[None, :]                      # [S, 64]
    cos_t, sin_t = np.cos(freqs), np.sin(freqs)
    ropeC = np.empty((DK, S), np.float32)
    ropeS = np.empty((DK, S), np.float32)
    ropeC[0::2] = cos_t.T
    ropeC[1::2] = cos_t.T
    ropeS[0::2] = -sin_t.T
    ropeS[1::2] = sin_t.T

    # diagonal causal masks: block m (of the 4 key blocks overlapping a
    # 512-query chunk) keeps kk <= qq - 128*m
    kk = np.arange(JB)[:, None]
    qq = np.arange(QCH)[None, :]
    maskd = np.concatenate(
        [np.where(kk <= qq - 128 * m, 0.0, MASK_NEG).astype(np.float32)
         for m in range(4)], axis=1)                           # [128, 4*512]

    ident = np.eye(128, dtype=np.float32).astype(np_dt)

    scale = 1.0 / np.sqrt(np.float32(DK))
    in_maps = []
    for c in range(N_CORES):
        rows = slice(c * HPC * DK, (c + 1) * HPC * DK)
        in_maps.append({
            "xT": xT,
            "wqT": np.ascontiguousarray((W_q[rows] * scale).T).astype(np_dt),
            "wkT": np.ascontiguousarray(W_k[rows].T).astype(np_dt),
            "wvT": np.ascontiguousarray(W_v[rows].T).astype(np_dt),
            "woE": woE,
            "ropeC": ropeC,
            "ropeS": ropeS,
            "maskd": maskd,
            "ident": ident,
        })
    return in_maps


def kernel(x, W_q, W_k, W_v, W_o):
    x = np.asarray(x, dtype=np.float32)
    W_q = np.asarray(W_q, dtype=np.float32)
    W_k = np.asarray(W_k, dtype=np.float32)
    W_v = np.asarray(W_v, dtype=np.float32)
    W_o = np.asarray(W_o, dtype=np.float32)

    if "nc" not in _CACHE:
        _CACHE["nc"] = _build_nc()
    nc = _CACHE["nc"]

    in_maps = _host_inputs(x, W_q, W_k, W_v, W_o)
    res = bass_utils.run_bass_kernel_spmd(
        nc, in_maps, core_ids=list(range(N_CORES)))

    # outT per core: [D, 512] fp32; columns = [b0 256 | b1a 128 | b1b 128]
    out = np.empty((B, S, D), np.float32)
    for c in range(N_CORES):
        oT = res.results[c]["outT"]                            # [D, 512]
        out[0, c * 256:(c + 1) * 256] = oT[:, 0:256].T
        out[1, c * 128:(c + 1) * 128] = oT[:, 256:384].T
        out[1, 1024 + c * 128:1024 + (c + 1) * 128] = oT[:, 384:512].T
    return out


# revision 72
# speedup vs baseline: 1.1959x; 1.1473x over previous
"""Causal self-attention with RoPE on 8 Trainium2 NeuronCores.

Sharding: tensor-parallel over heads (16 heads -> 2 per core) for
QKV projections, RoPE and attention; AllToAll re-shards the attention
output from head-sharded to token-sharded; the output projection then
runs token-parallel, so no all-reduce is needed.

The AllToAll is split six ways -- (head h) x (batch-0 | batch-1 first
half | batch-1 second half) -- with token ownership remapped so every
piece fires as soon as its attention chunks finish:
  core c owns batch-0 tokens [256c, 256c+256), batch-1 tokens
  [128c, 128c+128) and [1024+128c, 1024+128c+128).
The last collective moves only 0.25 MiB and lands while the PE is
still busy with earlier output-projection work, hiding the entire
communication cost.

Shapes (hardcoded): x [2, 2048, 2048], W_* [2048, 2048], 16 heads,
d_k = 128, fp32 in/out, bf16 on-chip.

On-chip dataflow per core (all matmuls via PE, contraction on the
partition axis):
  - xT chunks [128d x (16kb x 256t)] stream in; per head h:
      qT/kT [128dk, 256t] = sum_kb Wq_h_kb.T @ xT_kb   (PSUM)
      RoPE applied with a stream_shuffle pair-swap + 2 muls + add
  - v in natural [token, d] layout: v = x_blk @ Wv.T
  - attention works on transposed scores: ST[j*128 keys, 512 q] =
      kT_j.T @ qT_i ; p = exp(ST + causal_mask); outT += v_j.T @ p
      -- no max-subtraction needed (logits are O(1) by construction).
  - softmax denominator via transposed micro-matmuls: for each 128-q
    slice, lT[128q, 1] += p_slice.T @ ones -- output free size 1, so
    the PE streams ~0 columns (vs 512 for the ones.T @ p row-sum).
  - normalize: r = 1/l; [128,4] -> PE-transpose -> [4,128] -> 4 bcast
    matmuls -> R[128, 512]; y = outT * R
  - out projection per eb feature block: 16-step PSUM accumulation
    over all 16 y row-blocks, one pass per ownership segment.
"""

import sys

for _p in ("/opt/trn_rl_repo", "/opt/pypackages"):
    if _p not in sys.path:
        sys.path.insert(0, _p)

import numpy as np

import concourse.bass as bass
import concourse.bacc as bacc
import concourse.mybir as mybir
import concourse.tile as tile
from concourse import bass_utils
from concourse.alu_op_type import AluOpType

# ---------------------------------------------------------------- config
N_CORES = 8
B, S, D = 2, 2048, 2048
H = 16
DK = D // H              # 128
HPC = H // N_CORES       # 2 heads per core
TOK = B * S              # 4096
SUB = 256                # token sub-chunk for projections
QCH = 512                # attention query chunk
JB = 128                 # attention key block
NSUB = TOK // SUB        # 16
KB = D // 128            # 16 contraction blocks
ROPE_BASE = 10000.0
MASK_NEG = -30000.0
SEGW = (256, 128, 128)   # ownership segment widths (b0, b1a, b1b)

F32 = mybir.dt.float32
BF16 = mybir.dt.bfloat16
L_TRANSPOSED = True      # False: row-sum l via ones.T @ p (slower, simple)


def _np_dt():
    import ml_dtypes
    return np.dtype(ml_dtypes.bfloat16)


# ---------------------------------------------------------------- build
_CACHE = {}


def _build_nc(repeat=1):
    dt = BF16
    nc = bacc.Bacc("TRN2", target_bir_lowering=False, debug=False,
                   num_devices=N_CORES)

    xT = nc.dram_tensor("xT", [D, TOK], dt, kind="ExternalInput")
    wqT = nc.dram_tensor("wqT", [D, HPC * DK], dt, kind="ExternalInput")
    wkT = nc.dram_tensor("wkT", [D, HPC * DK], dt, kind="ExternalInput")
    wvT = nc.dram_tensor("wvT", [D, HPC * DK], dt, kind="ExternalInput")
    # eb-major Wo: woE[eb*128+p, dl*128+c] = W_o[eb*128+c, dl*128+p]
    woE = nc.dram_tensor("woE", [D, D], dt, kind="ExternalInput")
    ropeC = nc.dram_tensor("ropeC", [DK, S], dt, kind="ExternalInput")
    ropeS = nc.dram_tensor("ropeS", [DK, S], dt, kind="ExternalInput")
    maskd = nc.dram_tensor("maskd", [JB, 4 * QCH], dt, kind="ExternalInput")
    ident = nc.dram_tensor("ident", [128, 128], dt, kind="ExternalInput")
    outT = nc.dram_tensor("outT", [D, QCH], F32, kind="ExternalOutput")

    swap_mask = [i ^ 1 for i in range(32)]

    import contextlib
    with tile.TileContext(nc) as tc:
      for _rep in range(repeat):
        with contextlib.ExitStack() as st_outer:
            dram = st_outer.enter_context(
                tc.tile_pool(name="dram", bufs=1, space="DRAM"))
            # one collective per segment, both heads stacked: block c =
            # [h0 y rows | h1 y rows] for core c's tokens
            y_a2a = [dram.tile([N_CORES * HPC * 128, w], dt,
                               name=f"y_a2a{s}")
                     for s, w in enumerate(SEGW)]
            yfull = [dram.tile([N_CORES * HPC * 128, w], dt,
                               name=f"yfull{s}")
                     for s, w in enumerate(SEGW)]

            const = st_outer.enter_context(tc.tile_pool(name="const", bufs=1))
            # Wo + yT pools live alongside the pass pools (no address
            # reuse), so Wo prefetch can run during the head-1 pass.
            wopool = st_outer.enter_context(
                tc.tile_pool(name="wopool", bufs=1))
            ytpool = st_outer.enter_context(
                tc.tile_pool(name="ytpool", bufs=1))
            st_xq = st_outer.enter_context(contextlib.ExitStack())
            # xpool depth 5: the scalar queue dispatches xt quarters well
            # ahead of the PE; a deep ring keeps the ring-reuse WAR dep
            # pre-satisfied so it never head-of-line-blocks exp dispatch
            xpool = st_xq.enter_context(
                tc.tile_pool(name="xpool", bufs=5, side="right"))
            qpool = st_xq.enter_context(
                tc.tile_pool(name="qpool", bufs=2, side="right"))
            kvpool = st_xq.enter_context(
                tc.tile_pool(name="kvpool", bufs=8, side="right"))
            vpool = st_xq.enter_context(
                tc.tile_pool(name="vpool", bufs=20, side="right"))
            work = st_outer.enter_context(tc.tile_pool(name="work", bufs=2))
            ppool = st_outer.enter_context(tc.tile_pool(name="ppool", bufs=3))
            ps_proj = st_outer.enter_context(
                tc.tile_pool(name="ps_proj", bufs=2, space="PSUM"))
            ps_st = st_outer.enter_context(
                tc.tile_pool(name="ps_st", bufs=2, space="PSUM"))
            ps_out = st_outer.enter_context(
                tc.tile_pool(name="ps_out", bufs=1, space="PSUM"))
            ps_misc = st_outer.enter_context(
                tc.tile_pool(name="ps_misc", bufs=1, space="PSUM"))
            ps_lt = st_outer.enter_context(
                tc.tile_pool(name="ps_lt", bufs=1, space="PSUM"))

            def xt_dma(xt, sc):
                # xt quarters have no data deps, so they never head-of-
                # line-block exp dispatch (scalar) or y stores (sync).
                # Early chunks split across both queues: the dispatch
                # rate of one queue can't keep up with the projection-
                # only startup phase.
                for xh in range(4):
                    eng = nc.scalar if (sc > 5 or xh % 2 == 0) else nc.sync
                    xt_q(xt, sc, xh, eng)

            # ---- persistent constants in SBUF
            wq_sb = const.tile([128, KB * HPC * DK], dt)
            wk_sb = const.tile([128, KB * HPC * DK], dt)
            wv_sb = const.tile([128, KB * HPC * DK], dt)
            ropeC_sb = const.tile([DK, S], dt)
            ropeS_sb = const.tile([DK, S], dt)
            maskd_sb = const.tile([JB, 4 * QCH], dt)
            ident_sb = const.tile([128, 128], dt)

            def xt_q(xt, sc, xh, eng):
                KH = KB // 4
                eng.dma_start(
                    xt[:, xh * KH * SUB:(xh + 1) * KH * SUB]
                      .rearrange("p (kb t) -> p kb t", kb=KH),
                    xT.ap()[xh * KH * 128:(xh + 1) * KH * 128,
                            sc * SUB:(sc + 1) * SUB]
                      .rearrange("(kb p) t -> p kb t", p=128))

            def w_dma(sb_t, dr, ngrp, engs):
                GW = KB // ngrp
                for g in range(ngrp):
                    m0 = g * GW * HPC * DK
                    engs[g % len(engs)].dma_start(
                        sb_t[:, m0:m0 + GW * HPC * DK]
                            .rearrange("p (kb m) -> p kb m", kb=GW),
                        dr.ap()[g * GW * 128:(g + 1) * GW * 128, :]
                          .rearrange("(kb p) m -> p kb m", p=128))

            # startup dispatch order tuned for the first projections:
            # wq g0 leads the sync queue, xt0's first quarter leads
            # scalar; wk/wv/mask ride the otherwise-idle gpsimd queue
            xt0 = xpool.tile([128, KB * SUB], dt, tag="xt", name="xt")
            xt_q(xt0, 0, 0, nc.scalar)
            GQ = KB // 8
            def wq_grp(g):
                m0 = g * GQ * HPC * DK
                nc.sync.dma_start(
                    wq_sb[:, m0:m0 + GQ * HPC * DK]
                         .rearrange("p (kb m) -> p kb m", kb=GQ),
                    wqT.ap()[g * GQ * 128:(g + 1) * GQ * 128, :]
                       .rearrange("(kb p) m -> p kb m", p=128))
            for g in range(4):
                wq_grp(g)
            xt_q(xt0, 0, 1, nc.scalar)
            nc.sync.dma_start(ropeS_sb[:], ropeS[:])
            xt_q(xt0, 0, 2, nc.scalar)
            for g in range(4, 8):
                wq_grp(g)
            xt_q(xt0, 0, 3, nc.scalar)
            nc.scalar.dma_start(ropeC_sb[:], ropeC[:])
            GW = KB // 4
            gp_order = [(wk_sb, wkT, 0), (wv_sb, wvT, 0), (None, None, -1),
                        (wk_sb, wkT, 1), (wv_sb, wvT, 1),
                        (wk_sb, wkT, 2), (wv_sb, wvT, 2),
                        (wk_sb, wkT, 3), (wv_sb, wvT, 3)]
            for sb_t, dr, g in gp_order:
                if g < 0:
                    nc.gpsimd.dma_start(maskd_sb[:], maskd[:])
                    continue
                m0 = g * GW * HPC * DK
                nc.gpsimd.dma_start(
                    sb_t[:, m0:m0 + GW * HPC * DK]
                        .rearrange("p (kb m) -> p kb m", kb=GW),
                    dr.ap()[g * GW * 128:(g + 1) * GW * 128, :]
                      .rearrange("(kb p) m -> p kb m", p=128))
            nc.gpsimd.dma_start(ident_sb[:], ident[:])
            ones_col_f32 = const.tile([128, 1], F32)
            ones_row_f32 = const.tile([1, 128], F32)
            nc.vector.memset(ones_col_f32[:], 1.0)
            nc.vector.memset(ones_row_f32[:], 1.0)
            ones_col = const.tile([128, 1], dt)
            ones_row = const.tile([1, 128], dt)
            nc.vector.tensor_copy(ones_col[:], ones_col_f32[:])
            nc.vector.tensor_copy(ones_row[:], ones_row_f32[:])

            v_tiles = {}
            wo_tiles = [None] * KB
            # fused out-projection rhs tiles: batch 0 (seg 0) and batch 1
            # (segs 1|2 interleaved per dl block); loads are emitted right
            # after their collectives
            yT_b0 = ytpool.tile([128, KB * 256], dt, tag="ytb0", name="yt0")
            yT_b1 = ytpool.tile([128, KB * 256], dt, tag="ytb1", name="yt1")

            def rope_combine(ps_in, out_ap, s0, n):
                """out = ps_in * C + shuffle(ps_in) * S  (RoPE)."""
                qsh = work.tile([128, SUB], F32, tag="qsh")
                t1 = work.tile([128, SUB], F32, tag="t1")
                nc.vector.stream_shuffle(qsh[:, :n], ps_in, swap_mask)
                nc.any.tensor_tensor(
                    t1[:, :n], ps_in, ropeC_sb[:, s0:s0 + n], AluOpType.mult)
                nc.vector.tensor_tensor(
                    qsh[:, :n], qsh[:, :n], ropeS_sb[:, s0:s0 + n],
                    AluOpType.mult)
                nc.any.tensor_tensor(out_ap, t1[:, :n], qsh[:, :n],
                                     AluOpType.add)

            # ================= single pass over the sequence
            # Both heads' projections run per chunk; head-0 attention
            # fires on the chunk's completing sub-chunk, head-1's is
            # staggered one sub-chunk later so the two epilogues never
            # contend for the shared PSUM banks.
            qT_tiles = {}
            kT_tiles = {}

            def attn_chunk(h, b, i_q):
                    n_j = 4 * i_q + 4
                    qT = qT_tiles[(h, b, i_q)]
                    ps_o = ps_out.tile([128, QCH], F32, tag="att_out")
                    # all four 128-q-slice denominators live in one PSUM
                    # bank as four column accumulation groups
                    if L_TRANSPOSED:
                        ps_l4 = ps_lt.tile([128, 4], F32, tag="lt",
                                           name="ps_l4")
                    else:
                        ps_lrow = ps_lt.tile([1, QCH], F32, tag="lrow",
                                             name="ps_lrow")
                    # software-pipelined block loop: the scores matmul for
                    # block j+1 is emitted BEFORE block j's l/av consume
                    # p_j, so the PE never waits on the exp latency
                    def consume(j, p_t):
                        m = j - 4 * i_q
                        q0 = 128 * m if m > 0 else 0
                        if L_TRANSPOSED:
                            # one bank-wide accumulation group: start=True
                            # zeroes the whole 2KB PSUM bank, so only the
                            # first micro-matmul starts and only the very
                            # last one stops
                            for qs in range(max(m, 0), 4):
                                nc.tensor.matmul(
                                    ps_l4[:, qs:qs + 1],
                                    p_t[:, qs * 128:(qs + 1) * 128],
                                    ones_col[:],
                                    start=(j == 0 and qs == 0),
                                    stop=(j == n_j - 1 and qs == 3))
                        else:
                            nc.tensor.matmul(
                                ps_lrow[:, q0:], ones_col[:], p_t[:, q0:],
                                start=(j == 0), stop=(j == n_j - 1))
                        nc.tensor.matmul(
                            ps_o[:, q0:],
                            v_tiles[(b, j)][:, h * DK:(h + 1) * DK],
                            p_t[:, q0:],
                            start=(j == 0), stop=(j == n_j - 1))

                    pending = None
                    for j in range(n_j):
                        jc, jr = j // 4, j % 4
                        # diagonal blocks with offset m have their first
                        # 128*m query columns fully masked: shrink all the
                        # work to the valid column range. j == 0 is always
                        # full width, so it opens the PSUM groups.
                        m = j - 4 * i_q
                        q0 = 128 * m if m > 0 else 0
                        ps_s = ps_st.tile([JB, QCH], F32, tag="st")
                        nc.tensor.matmul(
                            ps_s[:, q0:],
                            kT_tiles[(h, b, jc)][:, jr * 128:(jr + 1) * 128],
                            qT[:, q0:],
                            start=True, stop=True)
                        p_t = ppool.tile([JB, QCH], dt, tag="p")
                        if m >= 0:
                            # diagonal block: the causal staircase only
                            # bites the single 128-query window [q0,
                            # q0+128) -- columns beyond it see all 128
                            # keys, columns before it aren't computed
                            nc.vector.tensor_tensor(
                                ps_s[:, q0:q0 + 128], ps_s[:, q0:q0 + 128],
                                maskd_sb[:, m * QCH + q0:m * QCH + q0 + 128],
                                AluOpType.add)
                        nc.scalar.activation(
                            p_t[:, q0:], ps_s[:, q0:],
                            mybir.ActivationFunctionType.Exp)
                        if pending is not None:
                            consume(*pending)
                        pending = (j, p_t)
                    consume(*pending)
                    # normalize: r = 1/l; four single-column PE
                    # transposes [128,1] -> [1,128] (outputs at
                    # partition 0), then one K=1 broadcast matmul
                    r_sb = work.tile([1, QCH], dt, tag="rrow")
                    if L_TRANSPOSED:
                        lr_sb = work.tile([128, 4], dt, tag="lr")
                        with nc.allow_low_precision(
                                reason="1/l bcast in bf16; y is bf16 anyway"):
                            nc.vector.reciprocal(lr_sb[:], ps_l4[:])
                        ps_row = ps_lt.tile([1, QCH], dt, tag="row",
                                            name="ps_row")
                        for qs in range(4):
                            # start=True zeroes the whole bank: only the
                            # first transpose starts, only the last stops
                            nc.tensor.matmul(ps_row[:, qs * 128:
                                                    (qs + 1) * 128],
                                             lr_sb[:, qs:qs + 1],
                                             ident_sb[:],
                                             is_transpose=True,
                                             start=(qs == 0),
                                             stop=(qs == 3))
                        nc.any.tensor_copy(r_sb[:], ps_row[:])
                    else:
                        with nc.allow_low_precision(
                                reason="1/l bcast in bf16; y is bf16 anyway"):
                            nc.vector.reciprocal(r_sb[:], ps_lrow[:])
                    ps_r = ps_misc.tile([128, QCH], F32, tag="R")
                    nc.tensor.matmul(ps_r[:], ones_row[:], r_sb[:],
                                     start=True, stop=True)
                    r_bc = work.tile([128, QCH], F32, tag="rbc")
                    nc.vector.tensor_copy(r_bc[:], ps_r[:])
                    y_sb = work.tile([128, QCH], dt, tag="y")
                    nc.vector.tensor_tensor(y_sb[:], ps_o[:], r_bc[:],
                                            AluOpType.mult)
                    # one batched store per (chunk, head), split by token
                    # ownership segment. Batch 0 rides the sync queue
                    # (which carries nothing else during the pass); batch
                    # 1 goes via gpsimd SWDGE, whose completion semaphore
                    # isn't shared with other queues' traffic, so the
                    # late collectives fire the moment their stores land.
                    if b == 0:
                        for t in range(2):
                            r0 = (2 * i_q + t) * 256 + h * 128
                            nc.sync.dma_start(
                                y_a2a[0][r0:r0 + 128, :],
                                y_sb[:, t * 256:(t + 1) * 256])
                    else:
                        seg = 1 if i_q < 2 else 2
                        for t in range(4):
                            r0 = (4 * (i_q % 2) + t) * 256 + h * 128
                            nc.gpsimd.dma_start(
                                y_a2a[seg][r0:r0 + 128, :],
                                y_sb[:, t * 128:(t + 1) * 128])

            for sc in range(NSUB):
                b = sc // (NSUB // B)
                s0 = (sc % (NSUB // B)) * SUB       # position within batch
                half = sc % 2
                i_q = (sc % (NSUB // B)) // 2       # query chunk in batch

                # head-1 attention for the chunk completed last sub-chunk,
                # emitted first so its collectives fire as early as possible
                if half == 0 and sc > 0:
                    pb = (sc - 1) // (NSUB // B)
                    pq = ((sc - 1) % (NSUB // B)) // 2
                    attn_chunk(1, pb, pq)
                    # both heads of the segment's chunks now stored:
                    # fire the combined AllToAll
                    seg = (0 if (pb, pq) == (0, 3) else
                           1 if (pb, pq) == (1, 1) else None)
                    if seg is not None:
                        nc.gpsimd.collective_compute(
                            "AllToAll", AluOpType.bypass,
                            replica_groups=[list(range(N_CORES))],
                            ins=[y_a2a[seg].opt()],
                            outs=[yfull[seg].opt()])
                    if seg == 1:
                        # batch-0 yT load, emitted at the sc12 boundary:
                        # by the time the sync queue reaches it the seg-0
                        # collective has long landed, so it dispatches
                        # immediately and blocks no y stores behind it
                        nc.sync.dma_start(
                            yT_b0.rearrange("p (dl t) -> p dl t", dl=KB),
                            yfull[0][:]
                            .rearrange("(dl p) t -> p dl t", p=128))

                # xt prefetch two sub-chunks ahead: the DMA dispatches
                # before the attention exp/dispatch backlog builds up on
                # the scalar queue
                if sc == 0:
                    xt_tiles = {0: xt0}
                    for pf in (1, 2):
                        xt_tiles[pf] = xpool.tile([128, KB * SUB], dt,
                                                  tag="xt", name="xt")
                        xt_dma(xt_tiles[pf], pf)
                elif sc + 2 < NSUB:
                    xt_tiles[sc + 2] = xpool.tile([128, KB * SUB], dt,
                                                  tag="xt", name="xt")
                    xt_dma(xt_tiles[sc + 2], sc + 2)
                xt = xt_tiles[sc]

                # Wo prefetch at sc 4-11: after the startup burst, but
                # done before the batch-1 y stores need the SWDGE ring
                if 4 <= sc <= 11:
                    for eb in (2 * (sc - 4), 2 * (sc - 4) + 1):
                        wo_eb = wopool.tile([128, KB * 128], dt,
                                            tag=f"wo{eb}", name="wo_eb")
                        nc.gpsimd.dma_start(
                            wo_eb[:], woE.ap()[eb * 128:(eb + 1) * 128, :])
                        wo_tiles[eb] = wo_eb

                # ---- q/k projections + rope, both heads
                for h in range(HPC):
                    if half == 0:
                        qT_tiles[(h, b, i_q)] = qpool.tile(
                            [128, QCH], dt, tag=f"qT{h}", name="qT")
                    if (h, b, i_q) not in kT_tiles:
                        kT_tiles[(h, b, i_q)] = kvpool.tile(
                            [128, QCH], dt, tag=f"kT{h}", name="kT")
                    for (w_sb, dst) in ((wq_sb, qT_tiles[(h, b, i_q)]),
                                        (wk_sb, kT_tiles[(h, b, i_q)])):
                        psq = ps_proj.tile([128, SUB], F32, tag="proj")
                        for kb in range(KB):
                            nc.tensor.matmul(
                                psq[:],
                                w_sb[:, kb * HPC * DK + h * DK:
                                     kb * HPC * DK + (h + 1) * DK],
                                xt[:, kb * SUB:(kb + 1) * SUB],
                                start=(kb == 0), stop=(kb == KB - 1))
                        rope_combine(psq[:],
                                     dst[:, half * SUB:(half + 1) * SUB],
                                     s0, SUB)

                # ---- v projection, both heads at once; the even sc's v
                # is deferred into the odd sc (before its attention needs
                # it) so the startup critical path doesn't wait on wv
                def v_proj(vsc, vxt):
                    for tb in range(SUB // 128):
                        jb_b = (vsc % (NSUB // B)) * 2 + tb
                        psv = ps_proj.tile([128, HPC * DK], F32,
                                           tag="proj", name="psv")
                        for kb in range(KB):
                            nc.tensor.matmul(
                                psv[:],
                                vxt[:, kb * SUB + tb * 128:
                                    kb * SUB + (tb + 1) * 128],
                                wv_sb[:, kb * HPC * DK:(kb + 1) * HPC * DK],
                                start=(kb == 0), stop=(kb == KB - 1))
                        vt = vpool.tile([128, HPC * DK], dt, tag="v",
                                        name="vt")
                        nc.vector.tensor_copy(vt[:], psv[:])
                        v_tiles[(vsc // (NSUB // B), jb_b)] = vt

                if half == 0:
                    prev_xt = xt
                else:
                    v_proj(sc - 1, prev_xt)
                    v_proj(sc, xt)

                # ---- head-0 attention for the completed query chunk
                if half == 1:
                    attn_chunk(0, b, i_q)

            # trailing head-1 attention for the last chunk + final a2a
            attn_chunk(1, B - 1, NSUB // B // 2 - 1)
            nc.sync.dma_start(
                yT_b1.rearrange("p (dl t) -> p dl t", dl=KB)[:, :, 0:128],
                yfull[1][:].rearrange("(dl p) t -> p dl t", p=128))
            nc.gpsimd.collective_compute(
                "AllToAll", AluOpType.bypass,
                replica_groups=[list(range(N_CORES))],
                ins=[y_a2a[2].opt()], outs=[yfull[2].opt()])
            nc.sync.dma_start(
                yT_b1.rearrange("p (dl t) -> p dl t", dl=KB)[:, :, 128:256],
                yfull[2][:].rearrange("(dl p) t -> p dl t", p=128))

            # x/q/kv/v pools are dead now; the out-projection reads the
            # AllToAll results (token-sharded y) against the resident Wo
            st_xq.close()

            obuf = st_outer.enter_context(tc.tile_pool(name="obuf", bufs=4))
            # ---- output projection, one fused pass per batch
            for bi, yt in enumerate((yT_b0, yT_b1)):
                for eb in range(KB):
                    ps_w = ps_st.tile([JB, QCH], F32, tag="st", name="ps_w")
                    for dl in range(KB):
                        nc.tensor.matmul(
                            ps_w[:, :256],
                            wo_tiles[eb][:, dl * 128:(dl + 1) * 128],
                            yt[:, dl * 256:(dl + 1) * 256],
                            start=(dl == 0), stop=(dl == KB - 1))
                    o_sb = obuf.tile([128, QCH], F32, tag="osb", name="o_sb")
                    nc.vector.tensor_copy(o_sb[:, :256], ps_w[:, :256])
                    nc.scalar.dma_start(
                        outT[eb * 128:(eb + 1) * 128,
                             bi * 256:(bi + 1) * 256],
                        o_sb[:, :256])

    nc.finalize()
    return nc


# ---------------------------------------------------------------- host
def _host_inputs(x, W_q, W_k, W_v, W_o):
    np_dt = _np_dt()
    xT = np.ascontiguousarray(
        x.reshape(TOK, D).T).astype(np_dt)                     # [D, TOK]
    # eb-major Wo for contiguous per-eb DMA rows:
    # woE[eb*128+p, dl*128+c] = W_o[eb*128+c, dl*128+p]
    woE = np.ascontiguousarray(
        W_o.reshape(KB, 128, KB, 128).transpose(0, 3, 2, 1)
        .reshape(D, D)).astype(np_dt)

    # RoPE tables, expanded to [DK, S] with interleaved pairs; the sign
    # table carries -sin on even rows, +sin on odd rows.
    i = np.arange(0, DK, 2, dtype=np.float32)
    theta = 1.0 / (ROPE_BASE ** (i / DK))                      # [64]
    pos = np.arange(S, dtype=np.float32)
    freqs = pos[:, None] * theta[None, :]                      # [S, 64]
    cos_t, sin_t = np.cos(freqs), np.sin(freqs)
    ropeC = np.empty((DK, S), np.float32)
    ropeS = np.empty((DK, S), np.float32)
    ropeC[0::2] = cos_t.T
    ropeC[1::2] = cos_t.T
    ropeS[0::2] = -sin_t.T
    ropeS[1::2] = sin_t.T
    ropeC = ropeC.astype(np_dt)
    ropeS = ropeS.astype(np_dt)

    # diagonal causal masks: block m (of the 4 key blocks overlapping a
    # 512-query chunk) keeps kk <= qq - 128*m
    kk = np.arange(JB)[:, None]
    qq = np.arange(QCH)[None, :]
    maskd = np.concatenate(
        [np.where(kk <= qq - 128 * m, 0.0, MASK_NEG).astype(np.float32)
         for m in range(4)], axis=1).astype(np_dt)             # [128, 4*512]

    ident = np.eye(128, dtype=np.float32).astype(np_dt)

    scale = 1.0 / np.sqrt(np.float32(DK))
    in_maps = []
    for c in range(N_CORES):
        rows = slice(c * HPC * DK, (c + 1) * HPC * DK)
        in_maps.append({
            "xT": xT,
            "wqT": np.ascontiguousarray((W_q[rows] * scale).T).astype(np_dt),
            "wkT": np.ascontiguousarray(W_k[rows].T).astype(np_dt),
            "wvT": np.ascontiguousarray(W_v[rows].T).astype(np_dt),
            "woE": woE,
            "ropeC": ropeC,
            "ropeS": ropeS,
            "maskd": maskd,
            "ident": ident,
        })
    return in_maps


def kernel(x, W_q, W_k, W_v, W_o):
    x = np.asarray(x, dtype=np.float32)
    W_q = np.asarray(W_q, dtype=np.float32)
    W_k = np.asarray(W_k, dtype=np.float32)
    W_v = np.asarray(W_v, dtype=np.float32)
    W_o = np.asarray(W_o, dtype=np.float32)

    if "nc" not in _CACHE:
        _CACHE["nc"] = _build_nc()
    nc = _CACHE["nc"]

    in_maps = _host_inputs(x, W_q, W_k, W_v, W_o)
    res = bass_utils.run_bass_kernel_spmd(
        nc, in_maps, core_ids=list(range(N_CORES)))

    # outT per core: [D, 512] fp32; columns = [b0 256 | b1a 128 | b1b 128]
    out = np.empty((B, S, D), np.float32)
    for c in range(N_CORES):
        oT = res.results[c]["outT"]                            # [D, 512]
        out[0, c * 256:(c + 1) * 256] = oT[:, 0:256].T
        out[1, c * 128:(c + 1) * 128] = oT[:, 256:384].T
        out[1, 1024 + c * 128:1024 + (c + 1) * 128] = oT[:, 384:512].T
    return out
